# revision 25
# baseline (speedup 1.0000x reference)
"""GCN (2-layer GraphConv, norm='both') on 8 Trainium2 NeuronCores.

Strategy (node-partition / in-edge pull, 2 dispatches):
  Dispatch 0 (conv0 + W1 fusion, bf16 matmuls):
    - Nodes in 8 contiguous shards; edges grouped by dst, chunked per 128 dsts.
    - Per chunk: indicator matmuls aggregate the host-laid-out 4-wide edge
      payload (3 od-scaled features + a 1/(ri*indeg) column that folds b0 in),
      then gT = W0p^T @ agg gives feat-major conv0 output, epilogue applies
      rod = ri*od and leaky (od folded inside: od*leaky(x) = leaky(od*x)),
      and hq = v @ W1 is computed per NODE (linearity: segsum(v[src]) @ W1 ==
      segsum((v@W1)[src])), written node-major in bf16.
  Dispatch 1 (conv1 = pure segment-sum of hq rows):
    - dma_gather (SWDGE) pulls hq[src] rows (256B bf16) for merged groups of
      chunks per call, per-chunk tile counts exact (trailing -1 idx are
      skipped by the Q7 ucode; only intra-chunk pad-to-128 costs descriptors).
    - Node-major indicator matmuls (lhsT=ind, rhs=x) segment-sum directly
      into [128 dst, D] PSUM; epilogue applies ri and b1.

All O(E*D)/O(N*D) compute and memory traffic runs on-device; the host does
index manipulation (sort/pad/relabel), normalization constants, and the
4-float-per-edge conv0 payload layout.
"""

import os
from contextlib import ExitStack

import numpy as np

import concourse.bass as bass
import concourse.tile as tile
from concourse import bacc, mybir
from concourse._compat import with_exitstack
from concourse.alu_op_type import AluOpType
from concourse.bass_utils import run_bass_kernel_spmd

F32 = mybir.dt.float32
BF16 = mybir.dt.bfloat16
I16 = mybir.dt.int16

NC_CORES = 8
D = 128
NEG_SLOPE = 0.01
GCHUNK = 1        # dst-chunks merged per dma_gather call
NQUEUES = 4       # SWDGE queues (disjoint Q7 cpu pairs -> parallel desc-gen)

LAST_EXEC_TIMES_NS: list = []
LAST_RESULTS: list = []


# --------------------------------------------------------------------------
# host-side prep
# --------------------------------------------------------------------------

def _wrap_idx(idx: np.ndarray) -> np.ndarray:
    """dma_gather index layout: position i lives at [i % 16, i // 16] of a
    16-row wrap, replicated 8x (one copy per Q7 core) -> [128, n/16] int16."""
    n = idx.shape[0]
    assert n % 16 == 0
    return np.tile(idx.astype(np.int16).reshape(n // 16, 16).T, (8, 1))


def _prep(src, dst, weight, significance, emb, W0, b0, W1, b1):
    n = weight.shape[0]
    e = src.shape[0]
    npc = n // NC_CORES
    assert npc * NC_CORES == n
    n_chunks = (npc + 127) // 128
    npad = n_chunks * 128
    half = (n + 1) // 2
    assert half <= 32767 and n - half <= 32767

    src = np.asarray(src).astype(np.int64)
    dst = np.asarray(dst).astype(np.int64)

    out_deg = np.bincount(src, minlength=n).astype(np.float64)
    in_deg = np.bincount(dst, minlength=n).astype(np.float64)
    od = (1.0 / np.sqrt(np.clip(out_deg, 1.0, None))).astype(np.float32)
    ri = (1.0 / np.sqrt(np.clip(in_deg, 1.0, None))).astype(np.float32)
    # b0 folding: payload col3 sums to 1/ri over each dst's in-edges
    with np.errstate(divide="ignore"):
        c_dst = np.where(in_deg > 0, 1.0 / (ri * np.maximum(in_deg, 1.0)), 0.0)
    c_dst = c_dst.astype(np.float32)

    emb_rows = np.asarray(emb, np.float32)[np.asarray(significance).astype(np.int64)]
    feats = np.concatenate(
        [np.asarray(weight, np.float32)[:, None], emb_rows], axis=1
    ) * od[:, None]                                        # [n, 3]

    order = np.argsort(dst, kind="stable")
    s_src, s_dst = src[order], dst[order]

    core_of = s_dst // npc
    loc = s_dst - core_of * npc
    chunk_of = loc // 128
    e_starts = np.searchsorted(core_of * n_chunks + chunk_of,
                               np.arange(NC_CORES * n_chunks + 1))

    # ---- conv0: per-chunk exact tile counts (max over cores) ----
    cnt_all = np.diff(e_starts).reshape(NC_CORES, n_chunks)
    t0k = np.maximum(1, -(-cnt_all.max(axis=0) // 128)).astype(np.int64)
    x0off = np.zeros(n_chunks + 1, np.int64)
    np.cumsum(t0k, out=x0off[1:])
    tot0 = int(x0off[-1])

    x0h = np.zeros((NC_CORES, 128, tot0 * 4), np.float32)
    dv0 = np.full((NC_CORES, 128, tot0), -1.0, np.float32)

    # ---- conv1: per (chunk,bucket) exact tile counts, shared across cores ----
    m0 = (s_src < half)
    # per (core, chunk, bucket) edge counts
    cnt_b = np.zeros((NC_CORES, n_chunks, 2), np.int64)
    for c in range(NC_CORES):
        for k in range(n_chunks):
            s0, s1 = e_starts[c * n_chunks + k], e_starts[c * n_chunks + k + 1]
            lo = int(np.count_nonzero(m0[s0:s1]))
            cnt_b[c, k, 0], cnt_b[c, k, 1] = lo, (s1 - s0) - lo
    # shared per-(chunk,bucket) tile count = max over cores
    tkb = np.maximum(1, -(-cnt_b.max(axis=0) // 128))      # [n_chunks, 2]

    n_groups = -(-n_chunks // GCHUNK)
    # per (group,bucket): total tiles, shared
    gslices = [list(range(g * GCHUNK, min((g + 1) * GCHUNK, n_chunks)))
               for g in range(n_groups)]
    Tg = np.array([[int(tkb[ks, b].sum()) for b in range(2)] for ks in gslices])
    Tmax = int(Tg.max())

    # host buffers for conv1 idx / dv (idx unwrapped, length Tmax*128/call)
    idx_w = np.zeros((NC_CORES, 2, n_groups, Tmax * 128), np.int64)
    # dv1 layout: for chunk k bucket b, tiles occupy columns
    # [dvoff[k,b], dvoff[k,b]+tkb[k,b]) each of width 128 (flattened later)
    dvoff = np.zeros((n_chunks, 2), np.int64)
    acc = 0
    for k in range(n_chunks):
        for b in range(2):
            dvoff[k, b] = acc
            acc += int(tkb[k, b])
    tot_tiles = acc

    dv1 = np.full((NC_CORES, 128, tot_tiles), -1.0, np.float32)

    for c in range(NC_CORES):
        for k in range(n_chunks):
            s0, s1 = e_starts[c * n_chunks + k], e_starts[c * n_chunks + k + 1]
            es, ed = s_src[s0:s1], s_dst[s0:s1]
            dloc = (ed - c * npc - k * 128).astype(np.float32)

            # conv0 payload (col3 = c_dst of the edge's dst)
            ntk = int(t0k[k])
            n0k = ntk * 128
            o0 = int(x0off[k])
            pay = np.zeros((n0k, 4), np.float32)
            nreal = s1 - s0
            pay[:nreal, :3] = feats[es]
            pay[:nreal, 3] = c_dst[ed]
            x0h[c, :, o0 * 4:(o0 + ntk) * 4] = (
                pay.reshape(ntk, 128, 4).transpose(1, 0, 2).reshape(128, ntk * 4)
            )
            dvc = np.full(n0k, -1.0, np.float32)
            dvc[:nreal] = dloc
            dv0[c, :, o0:o0 + ntk] = dvc.reshape(ntk, 128).T

            # conv1 per bucket
            msk = m0[s0:s1]
            for b in range(2):
                sel = msk if b == 0 else ~msk
                sb = es[sel] - (0 if b == 0 else half)
                db = dloc[sel]
                nt = int(tkb[k, b])
                nb = nt * 128
                g = k // GCHUNK
                # pad with valid index 0 (indicator is -1 there, so the
                # gathered row contributes nothing)
                pidx = np.zeros(nb, np.int64)
                pidx[:sb.shape[0]] = sb
                dvb = np.full(nb, -1.0, np.float32)
                dvb[:db.shape[0]] = db
                o = dvoff[k, b]
                dv1[c, :, o:o + nt] = dvb.reshape(nt, 128).T
                toff = int(sum(int(tkb[kk, b]) for kk in gslices[g] if kk < k))
                idx_w[c, b, g, toff * 128:toff * 128 + nb] = pidx

    # wrap conv1 idx: per (bucket, group) one gather call of Tg[g,b]*128 idx,
    # padded (with trailing -1) to Tmax*128 so shapes are shared
    idxh = np.zeros((NC_CORES, 2, n_groups, 128, Tmax * 8), np.int16)
    for c in range(NC_CORES):
        for b in range(2):
            for g in range(n_groups):
                idxh[c, b, g] = _wrap_idx(idx_w[c, b, g])

    vcnt = np.zeros((NC_CORES, 2 * n_groups), np.int32)
    for c in range(NC_CORES):
        for b in range(2):
            for g in range(n_groups):
                vcnt[c, b * n_groups + g] = int(
                    sum(int(cnt_b[c, kk, b]) for kk in gslices[g]))

    def _pc(v):
        out = np.ones((NC_CORES, 128, n_chunks), np.float32)
        for c in range(NC_CORES):
            vv = np.ones(npad, np.float32)
            vv[:npc] = v[c * npc:(c + 1) * npc]
            out[c] = vv.reshape(n_chunks, 128).T
        return out

    # rod row (feat-major epilogue): rod[n] = ri[n]*od[n], replicated 128 part
    rod = (ri * od).astype(np.float32)
    rodT = np.ones((NC_CORES, 128, npad), np.float32)
    for c in range(NC_CORES):
        vv = np.ones(npad, np.float32)
        vv[:npc] = rod[c * npc:(c + 1) * npc]
        rodT[c] = np.tile(vv[None, :], (128, 1))

    W0p4 = np.zeros((4, D), np.float32)
    W0p4[:3] = np.asarray(W0, np.float32)
    W0p4[3] = np.asarray(b0, np.float32)

    consts = {
        "iota_bc": np.tile(np.arange(128, dtype=np.float32)[None, :], (128, 1)),
        "b1_bc": np.tile(np.asarray(b1, np.float32)[None, :], (128, 1)),
        "W0p4": W0p4,
        "W1": np.asarray(W1, np.float32),
    }
    return dict(
        n=n, e=e, npc=npc, n_chunks=n_chunks, npad=npad, half=half,
        t0k=t0k, x0off=x0off, tot0=tot0,
        tkb=tkb, dvoff=dvoff, tot_tiles=tot_tiles, n_groups=n_groups,
        gslices=gslices, Tmax=Tmax,
        od=od, ri=ri, in_deg=in_deg, ri_pc=_pc(ri), rodT=rodT, vcnt=vcnt,
        x0h=x0h, dv0=dv0, dv1=dv1, idxh=idxh, consts=consts,
        b0=np.asarray(b0, np.float32), W1f=np.asarray(W1, np.float32),
    )


# --------------------------------------------------------------------------
# device programs
# --------------------------------------------------------------------------

def _new_nc(nq=1):
    return bacc.Bacc("TRN2", target_bir_lowering=False, debug=False,
                     num_devices=NC_CORES, num_swdge_queues=nq)


@with_exitstack
def _conv0_body(ctx: ExitStack, tc, aps, n_chunks, t0k, x0off, tot0):
    nc = tc.nc
    cpool = ctx.enter_context(tc.tile_pool(name="consts", bufs=1))
    pool = ctx.enter_context(tc.tile_pool(name="work", bufs=3))
    epool = ctx.enter_context(tc.tile_pool(name="epi", bufs=3))
    ps_a = ctx.enter_context(tc.tile_pool(name="ps_a", bufs=2, space="PSUM"))
    ps_g = ctx.enter_context(tc.tile_pool(name="ps_g", bufs=2, space="PSUM"))
    ps_q = ctx.enter_context(tc.tile_pool(name="ps_q", bufs=2, space="PSUM"))

    iota_sb = cpool.tile([128, 128], BF16)
    nc.sync.dma_start(iota_sb[:], aps["iota_bc"][:])
    w0_sb = cpool.tile([4, D], BF16)
    nc.sync.dma_start(w0_sb[:], aps["W0p4"][:])
    w1_sb = cpool.tile([D, D], BF16)
    nc.sync.dma_start(w1_sb[:], aps["W1"][:])
    dv0_sb = cpool.tile([128, tot0], BF16)
    nc.sync.dma_start(dv0_sb[:], aps["dv0"][:])
    rod_sb = cpool.tile([128, n_chunks * 128], F32)
    nc.sync.dma_start(rod_sb[:], aps["rodT"][:])
    x0_sb = cpool.tile([128, tot0 * 4], BF16)
    nc.sync.dma_start(x0_sb[:], aps["x0h"][:])
    hq_d = aps["hq"]        # [n_chunks*128, D] bf16 node-major output

    t0max = int(max(int(t) for t in t0k))
    for k in range(n_chunks):
        ntk = int(t0k[k])
        o0 = int(x0off[k])
        x0_k = x0_sb[:, o0 * 4:(o0 + ntk) * 4]
        ind_sb = pool.tile([128, t0max * 128], BF16, tag="ind")
        nc.vector.tensor_tensor(
            ind_sb[:, :ntk * 128].rearrange("p (t j) -> p t j", j=128),
            dv0_sb[:, o0:o0 + ntk].unsqueeze(2).broadcast_to([128, ntk, 128]),
            iota_sb[:].unsqueeze(1).broadcast_to([128, ntk, 128]),
            AluOpType.is_equal,
        )
        agg_ps = ps_a.tile([4, 128], F32, tag="agg")
        for t in range(ntk):
            nc.tensor.matmul(
                agg_ps[:],
                lhsT=x0_k[:, bass.ts(t, 4)],
                rhs=ind_sb[:, bass.ts(t, 128)],
                start=(t == 0),
                stop=(t == ntk - 1),
            )
        agg_sb = epool.tile([4, 128], BF16, tag="aggsb")
        nc.scalar.activation(agg_sb[:], agg_ps[:],
                             mybir.ActivationFunctionType.Copy)

        # gT[f, n] = W0p4^T @ agg  (+ b0 folded via payload col3)
        g_ps = ps_g.tile([128, 128], F32, tag="g")
        nc.tensor.matmul(g_ps[:], lhsT=w0_sb[:], rhs=agg_sb[:],
                         start=True, stop=True)

        # v = leaky(gT * rod)   (od folded: od*leaky(x) == leaky(od*x))
        z_sb = epool.tile([128, 128], F32, tag="z")
        nc.vector.tensor_tensor(z_sb[:], g_ps[:],
                                rod_sb[:, k * 128:(k + 1) * 128],
                                AluOpType.mult)
        v_sb = epool.tile([128, 128], BF16, tag="v")
        nc.vector.scalar_tensor_tensor(
            v_sb[:], z_sb[:], float(NEG_SLOPE), z_sb[:],
            AluOpType.mult, AluOpType.max,
        )

        # hq[n, f'] = v^T @ W1  (node-major)
        q_ps = ps_q.tile([128, D], F32, tag="q")
        nc.tensor.matmul(q_ps[:], lhsT=v_sb[:], rhs=w1_sb[:],
                         start=True, stop=True)
        hq_sb = epool.tile([128, D], BF16, tag="hq")
        nc.scalar.activation(hq_sb[:], q_ps[:],
                             mybir.ActivationFunctionType.Copy)
        nc.sync.dma_start(hq_d[k * 128:(k + 1) * 128, :], hq_sb[:])


@with_exitstack
def _conv1_body(ctx: ExitStack, tc, aps, p):
    nc = tc.nc
    n, half = p["n"], p["half"]
    n_chunks, tkb, dvoff = p["n_chunks"], p["tkb"], p["dvoff"]
    n_groups, gslices, Tmax = p["n_groups"], p["gslices"], p["Tmax"]

    cpool = ctx.enter_context(tc.tile_pool(name="consts", bufs=1))
    xpool = ctx.enter_context(tc.tile_pool(name="x", bufs=6))
    ipool = ctx.enter_context(tc.tile_pool(name="ind", bufs=3))
    epool = ctx.enter_context(tc.tile_pool(name="epi", bufs=4))
    ps_o = ctx.enter_context(tc.tile_pool(name="ps_o", bufs=4, space="PSUM"))

    iota_sb = cpool.tile([128, 128], BF16)
    nc.sync.dma_start(iota_sb[:], aps["iota_bc"][:])
    b1_sb = cpool.tile([128, 128], F32)
    nc.sync.dma_start(b1_sb[:], aps["b1_bc"][:])
    ri_sb = cpool.tile([128, n_chunks], F32)
    nc.sync.dma_start(ri_sb[:], aps["ri_pc"][:])
    dv1_sb = cpool.tile([128, p["tot_tiles"]], BF16)
    nc.sync.dma_start(dv1_sb[:], aps["dv1"][:])
    idx_sb = cpool.tile([128, 2 * n_groups * Tmax * 8], I16)
    nc.sync.dma_start(idx_sb[:], aps["idxh"][:])


    hq_d = aps["hq"]        # [n, D] bf16
    out_d = aps["out"]      # [n_chunks*128, D] f32

    qrr = 0
    for g in range(n_groups):
        ks = gslices[g]
        xb = []
        for b in range(2):
            Tg = int(sum(int(tkb[k, b]) for k in ks))
            x_sb = xpool.tile([128, Tmax * D], BF16, tag=f"x{b}")
            src_rows = hq_d[0:half, :] if b == 0 else hq_d[half:n, :]
            nidx = Tg * 128
            nc.gpsimd.dma_gather(
                out_ap=x_sb[:, :Tg * D].rearrange("p (t f) -> p t f", f=D),
                in_ap=src_rows,
                idxs_ap=idx_sb[:, (b * n_groups + g) * Tmax * 8:
                               (b * n_groups + g) * Tmax * 8 + Tg * 8],
                num_idxs=nidx,
                num_idxs_reg=nidx,
                elem_size=D,
                single_packet=(nidx // 16 <= 63),
                queue_num=qrr % NQUEUES,
            )
            qrr += 1
            xb.append(x_sb)

        for k in ks:
            nt_a, nt_b = int(tkb[k, 0]), int(tkb[k, 1])
            o_a, o_b = int(dvoff[k, 0]), int(dvoff[k, 1])
            nt = nt_a + nt_b
            ind_sb = ipool.tile([128, nt * 128], BF16, tag="ind")
            # indicator for bucket-a tiles then bucket-b tiles of this chunk
            nc.vector.tensor_tensor(
                ind_sb[:, :nt_a * 128].rearrange("p (t j) -> p t j", j=128),
                dv1_sb[:, o_a:o_a + nt_a].unsqueeze(2)
                    .broadcast_to([128, nt_a, 128]),
                iota_sb[:].unsqueeze(1).broadcast_to([128, nt_a, 128]),
                AluOpType.is_equal,
            )
            nc.vector.tensor_tensor(
                ind_sb[:, nt_a * 128:].rearrange("p (t j) -> p t j", j=128),
                dv1_sb[:, o_b:o_b + nt_b].unsqueeze(2)
                    .broadcast_to([128, nt_b, 128]),
                iota_sb[:].unsqueeze(1).broadcast_to([128, nt_b, 128]),
                AluOpType.is_equal,
            )
            # node-major segment-sum: o_ps[128 dst, D] accumulates
            o_ps = ps_o.tile([128, D], F32, tag="o")
            # tile offsets of this chunk within the group's x buffers
            ta0 = int(sum(int(tkb[kk, 0]) for kk in ks if kk < k))
            tb0 = int(sum(int(tkb[kk, 1]) for kk in ks if kk < k))
            mm = 0
            for t in range(nt_a):
                nc.tensor.matmul(
                    o_ps[:],
                    lhsT=ind_sb[:, bass.ts(t, 128)],
                    rhs=xb[0][:, bass.ts(ta0 + t, D)],
                    start=(mm == 0), stop=(mm == nt - 1),
                )
                mm += 1
            for t in range(nt_b):
                nc.tensor.matmul(
                    o_ps[:],
                    lhsT=ind_sb[:, bass.ts(nt_a + t, 128)],
                    rhs=xb[1][:, bass.ts(tb0 + t, D)],
                    start=(mm == 0), stop=(mm == nt - 1),
                )
                mm += 1

            out_sb = epool.tile([128, D], F32, tag="outsb")
            nc.vector.scalar_tensor_tensor(
                out_sb[:], o_ps[:], ri_sb[:, k:k + 1], b1_sb[:],
                AluOpType.mult, AluOpType.add,
            )
            nc.sync.dma_start(out_d[k * 128:(k + 1) * 128, :], out_sb[:])


def tensor_specs0(p):
    n_chunks, tot0 = p["n_chunks"], p["tot0"]
    return {
        "iota_bc": ((128, 128), BF16, "ExternalInput"),
        "W0p4": ((4, D), BF16, "ExternalInput"),
        "W1": ((D, D), BF16, "ExternalInput"),
        "dv0": ((128, tot0), BF16, "ExternalInput"),
        "rodT": ((128, n_chunks * 128), F32, "ExternalInput"),
        "x0h": ((128, tot0 * 4), BF16, "ExternalInput"),
        "hq": ((n_chunks * 128, D), BF16, "ExternalOutput"),
    }


def tensor_specs1(p):
    n, n_chunks = p["n"], p["n_chunks"]
    return {
        "iota_bc": ((128, 128), BF16, "ExternalInput"),
        "b1_bc": ((128, 128), F32, "ExternalInput"),
        "ri_pc": ((128, n_chunks), F32, "ExternalInput"),
        "dv1": ((128, p["tot_tiles"]), BF16, "ExternalInput"),
        "idxh": ((128, 2 * p["n_groups"] * p["Tmax"] * 8), I16, "ExternalInput"),
        "hq": ((n, D), BF16, "ExternalInput"),
        "out": ((n_chunks * 128, D), F32, "ExternalOutput"),
    }


def in_maps0(p):
    c = p["consts"]
    return [
        {"iota_bc": c["iota_bc"], "W0p4": c["W0p4"], "W1": c["W1"],
         "dv0": p["dv0"][i], "rodT": p["rodT"][i], "x0h": p["x0h"][i]}
        for i in range(NC_CORES)
    ]


def in_maps1(p, hq_full):
    c = p["consts"]
    return [
        {"iota_bc": c["iota_bc"], "b1_bc": c["b1_bc"], "ri_pc": p["ri_pc"][i],
         "dv1": p["dv1"][i],
         "idxh": np.ascontiguousarray(
             p["idxh"][i].reshape(2 * p["n_groups"], 128, p["Tmax"] * 8)
             .transpose(1, 0, 2).reshape(128, -1)),
         "hq": hq_full}
        for i in range(NC_CORES)
    ]


def _build(body, tensors, nq=1, **kw):
    nc = _new_nc(nq)
    aps = {
        name: nc.dram_tensor(name, list(shape), dtype, kind=kind).ap()
        for name, (shape, dtype, kind) in tensors.items()
    }
    with tile.TileContext(nc) as tc:
        body(tc, aps, **kw)
    nc.compile()
    return nc


class _SimResults:
    def __init__(self, results):
        self.results = results
        self.exec_time_ns = None


def _run_sim(nc, in_maps, out_names):
    from concourse.bass_interp import CoreSim
    results = []
    for im in in_maps:
        sim = CoreSim(nc)
        for k, v in im.items():
            sim.tensor(k)[:] = v
        sim.simulate(check_with_hw=False)
        results.append({k: np.array(sim.tensor(k)) for k in out_names})
    return _SimResults(results)


# --------------------------------------------------------------------------
# entry point
# --------------------------------------------------------------------------

def _to_bf16(a):
    import ml_dtypes
    return np.asarray(a).astype(ml_dtypes.bfloat16)


def kernel(src, dst, weight, significance, emb, W0, b0, W1, b1):
    global LAST_EXEC_TIMES_NS, LAST_RESULTS
    LAST_EXEC_TIMES_NS = []
    LAST_RESULTS = []
    trace = bool(os.environ.get("BASS_TRACE"))

    p = _prep(src, dst, weight, significance, emb, W0, b0, W1, b1)
    n, npc, n_chunks = p["n"], p["npc"], p["n_chunks"]
    c = p["consts"]

    # bf16-cast inputs where specs say BF16
    c["iota_bc"] = _to_bf16(c["iota_bc"])
    c["W0p4"] = _to_bf16(c["W0p4"])
    c["W1"] = _to_bf16(c["W1"])
    p["dv0"] = _to_bf16(p["dv0"])
    p["x0h"] = _to_bf16(p["x0h"])
    p["dv1"] = _to_bf16(p["dv1"])

    use_sim = bool(os.environ.get("BASS_SIM"))

    nc0 = _build(_conv0_body, tensor_specs0(p), n_chunks=n_chunks,
                 t0k=p["t0k"], x0off=p["x0off"], tot0=p["tot0"])
    if use_sim:
        res0 = _run_sim(nc0, in_maps0(p), ["hq"])
    else:
        res0 = run_bass_kernel_spmd(nc0, in_maps0(p),
                                    core_ids=list(range(NC_CORES)), trace=trace)
    LAST_RESULTS.append(res0)
    LAST_EXEC_TIMES_NS.append(res0.exec_time_ns)
    hq_full = np.concatenate(
        [np.asarray(res0.results[i]["hq"][:npc]) for i in range(NC_CORES)], axis=0
    )
    assert hq_full.shape == (n, D)

    # patch zero-in-degree nodes (b0 fold drops them): hq = leaky(b0)*od @ W1
    zid = np.nonzero(p["in_deg"] == 0)[0]
    if zid.size:
        b0f = p["b0"]
        v = np.where(b0f > 0, b0f, NEG_SLOPE * b0f)[None, :] * p["od"][zid][:, None]
        hq_full[zid] = _to_bf16(v @ p["W1f"])

    nc1 = _build(_conv1_body, tensor_specs1(p), nq=NQUEUES, p=p)
    if use_sim:
        res1 = _run_sim(nc1, in_maps1(p, hq_full), ["out"])
    else:
        res1 = run_bass_kernel_spmd(nc1, in_maps1(p, hq_full),
                                    core_ids=list(range(NC_CORES)), trace=trace)
    LAST_RESULTS.append(res1)
    LAST_EXEC_TIMES_NS.append(res1.exec_time_ns)

    out = np.concatenate(
        [res1.results[i]["out"][:npc] for i in range(NC_CORES)], axis=0
    )
    assert out.shape == (n, D)
    return out.astype(np.float32)


# revision 28
# speedup vs baseline: 1.1911x; 1.1911x over previous
"""GCN (2-layer GraphConv, norm='both') on 8 Trainium2 NeuronCores.

Strategy (node-partition / in-edge pull, 2 dispatches):
  Dispatch 0 (conv0 + W1 fusion, bf16 matmuls):
    - Nodes in 8 contiguous shards; edges grouped by dst, chunked per 128 dsts.
    - Per chunk: indicator matmuls aggregate the host-laid-out 4-wide edge
      payload (3 od-scaled features + a 1/(ri*indeg) column that folds b0 in),
      then gT = W0p^T @ agg gives feat-major conv0 output, epilogue applies
      rod = ri*od and leaky (od folded inside: od*leaky(x) = leaky(od*x)),
      and hq = v @ W1 is computed per NODE (linearity: segsum(v[src]) @ W1 ==
      segsum((v@W1)[src])), written node-major in bf16.
  Dispatch 1 (conv1 = pure segment-sum of hq rows):
    - dma_gather (SWDGE) pulls hq[src] rows (256B bf16) for merged groups of
      chunks per call, per-chunk tile counts exact (trailing -1 idx are
      skipped by the Q7 ucode; only intra-chunk pad-to-128 costs descriptors).
    - Node-major indicator matmuls (lhsT=ind, rhs=x) segment-sum directly
      into [128 dst, D] PSUM; epilogue applies ri and b1.

All O(E*D)/O(N*D) compute and memory traffic runs on-device; the host does
index manipulation (sort/pad/relabel), normalization constants, and the
4-float-per-edge conv0 payload layout.
"""

import os
from contextlib import ExitStack

import numpy as np

import concourse.bass as bass
import concourse.tile as tile
from concourse import bacc, mybir
from concourse._compat import with_exitstack
from concourse.alu_op_type import AluOpType
from concourse.bass_utils import run_bass_kernel_spmd

F32 = mybir.dt.float32
BF16 = mybir.dt.bfloat16
I16 = mybir.dt.int16

NC_CORES = 8
D = 128
NEG_SLOPE = 0.01
GCHUNK = 1        # dst-chunks merged per dma_gather call
NQUEUES = 4       # SWDGE queues (disjoint Q7 cpu pairs -> parallel desc-gen)

LAST_EXEC_TIMES_NS: list = []
LAST_RESULTS: list = []


# --------------------------------------------------------------------------
# host-side prep
# --------------------------------------------------------------------------

def _wrap_idx(idx: np.ndarray) -> np.ndarray:
    """dma_gather index layout: position i lives at [i % 16, i // 16] of a
    16-row wrap, replicated 8x (one copy per Q7 core) -> [128, n/16] int16."""
    n = idx.shape[0]
    assert n % 16 == 0
    return np.tile(idx.astype(np.int16).reshape(n // 16, 16).T, (8, 1))


def _prep(src, dst, weight, significance, emb, W0, b0, W1, b1):
    n = weight.shape[0]
    e = src.shape[0]
    npc = n // NC_CORES
    assert npc * NC_CORES == n
    n_chunks = (npc + 127) // 128
    npad = n_chunks * 128
    half = (n + 1) // 2
    assert half <= 32767 and n - half <= 32767

    src = np.asarray(src).astype(np.int64)
    dst = np.asarray(dst).astype(np.int64)

    out_deg = np.bincount(src, minlength=n).astype(np.float64)
    in_deg = np.bincount(dst, minlength=n).astype(np.float64)
    od = (1.0 / np.sqrt(np.clip(out_deg, 1.0, None))).astype(np.float32)
    ri = (1.0 / np.sqrt(np.clip(in_deg, 1.0, None))).astype(np.float32)
    # b0 folding: payload col3 sums to 1/ri over each dst's in-edges
    with np.errstate(divide="ignore"):
        c_dst = np.where(in_deg > 0, 1.0 / (ri * np.maximum(in_deg, 1.0)), 0.0)
    c_dst = c_dst.astype(np.float32)

    emb_rows = np.asarray(emb, np.float32)[np.asarray(significance).astype(np.int64)]
    feats = np.concatenate(
        [np.asarray(weight, np.float32)[:, None], emb_rows], axis=1
    ) * od[:, None]                                        # [n, 3]

    order = np.argsort(dst, kind="stable")
    s_src, s_dst = src[order], dst[order]

    core_of = s_dst // npc
    loc = s_dst - core_of * npc
    chunk_of = loc // 128
    e_starts = np.searchsorted(core_of * n_chunks + chunk_of,
                               np.arange(NC_CORES * n_chunks + 1))

    # ---- conv0: per-chunk exact tile counts (max over cores) ----
    cnt_all = np.diff(e_starts).reshape(NC_CORES, n_chunks)
    t0k = np.maximum(1, -(-cnt_all.max(axis=0) // 128)).astype(np.int64)
    x0off = np.zeros(n_chunks + 1, np.int64)
    np.cumsum(t0k, out=x0off[1:])
    tot0 = int(x0off[-1])

    x0h = np.zeros((NC_CORES, 128, tot0 * 4), np.float32)
    dv0 = np.full((NC_CORES, 128, tot0), -1.0, np.float32)

    # ---- conv1: per (chunk,bucket) exact tile counts, shared across cores ----
    m0 = (s_src < half)
    # per (core, chunk, bucket) edge counts
    cnt_b = np.zeros((NC_CORES, n_chunks, 2), np.int64)
    for c in range(NC_CORES):
        for k in range(n_chunks):
            s0, s1 = e_starts[c * n_chunks + k], e_starts[c * n_chunks + k + 1]
            lo = int(np.count_nonzero(m0[s0:s1]))
            cnt_b[c, k, 0], cnt_b[c, k, 1] = lo, (s1 - s0) - lo
    # shared per-(chunk,bucket) tile count = max over cores
    tkb = np.maximum(1, -(-cnt_b.max(axis=0) // 128))      # [n_chunks, 2]

    n_groups = -(-n_chunks // GCHUNK)
    # per (group,bucket): total tiles, shared
    gslices = [list(range(g * GCHUNK, min((g + 1) * GCHUNK, n_chunks)))
               for g in range(n_groups)]
    Tg = np.array([[int(tkb[ks, b].sum()) for b in range(2)] for ks in gslices])
    Tmax = int(Tg.max())

    # host buffers for conv1 idx / dv (idx unwrapped, length Tmax*128/call)
    idx_w = np.zeros((NC_CORES, 2, n_groups, Tmax * 128), np.int64)
    # dv1 layout: for chunk k bucket b, tiles occupy columns
    # [dvoff[k,b], dvoff[k,b]+tkb[k,b]) each of width 128 (flattened later)
    dvoff = np.zeros((n_chunks, 2), np.int64)
    acc = 0
    for k in range(n_chunks):
        for b in range(2):
            dvoff[k, b] = acc
            acc += int(tkb[k, b])
    tot_tiles = acc

    dv1 = np.full((NC_CORES, 128, tot_tiles), -1.0, np.float32)

    for c in range(NC_CORES):
        for k in range(n_chunks):
            s0, s1 = e_starts[c * n_chunks + k], e_starts[c * n_chunks + k + 1]
            es, ed = s_src[s0:s1], s_dst[s0:s1]
            dloc = (ed - c * npc - k * 128).astype(np.float32)

            # conv0 payload (col3 = c_dst of the edge's dst)
            ntk = int(t0k[k])
            n0k = ntk * 128
            o0 = int(x0off[k])
            pay = np.zeros((n0k, 4), np.float32)
            nreal = s1 - s0
            pay[:nreal, :3] = feats[es]
            pay[:nreal, 3] = c_dst[ed]
            x0h[c, :, o0 * 4:(o0 + ntk) * 4] = (
                pay.reshape(ntk, 128, 4).transpose(1, 0, 2).reshape(128, ntk * 4)
            )
            dvc = np.full(n0k, -1.0, np.float32)
            dvc[:nreal] = dloc
            dv0[c, :, o0:o0 + ntk] = dvc.reshape(ntk, 128).T

            # conv1 per bucket
            msk = m0[s0:s1]
            for b in range(2):
                sel = msk if b == 0 else ~msk
                sb = es[sel] - (0 if b == 0 else half)
                db = dloc[sel]
                nt = int(tkb[k, b])
                nb = nt * 128
                g = k // GCHUNK
                # pad with valid index 0 (indicator is -1 there, so the
                # gathered row contributes nothing)
                pidx = np.zeros(nb, np.int64)
                pidx[:sb.shape[0]] = sb
                dvb = np.full(nb, -1.0, np.float32)
                dvb[:db.shape[0]] = db
                o = dvoff[k, b]
                dv1[c, :, o:o + nt] = dvb.reshape(nt, 128).T
                toff = int(sum(int(tkb[kk, b]) for kk in gslices[g] if kk < k))
                idx_w[c, b, g, toff * 128:toff * 128 + nb] = pidx

    # wrap conv1 idx: per (bucket, group) one gather call of Tg[g,b]*128 idx,
    # padded (with trailing -1) to Tmax*128 so shapes are shared
    idxh = np.zeros((NC_CORES, 2, n_groups, 128, Tmax * 8), np.int16)
    for c in range(NC_CORES):
        for b in range(2):
            for g in range(n_groups):
                idxh[c, b, g] = _wrap_idx(idx_w[c, b, g])

    vcnt = np.zeros((NC_CORES, 2 * n_groups), np.int32)
    for c in range(NC_CORES):
        for b in range(2):
            for g in range(n_groups):
                vcnt[c, b * n_groups + g] = int(
                    sum(int(cnt_b[c, kk, b]) for kk in gslices[g]))

    def _pc(v):
        out = np.ones((NC_CORES, 128, n_chunks), np.float32)
        for c in range(NC_CORES):
            vv = np.ones(npad, np.float32)
            vv[:npc] = v[c * npc:(c + 1) * npc]
            out[c] = vv.reshape(n_chunks, 128).T
        return out

    # rod row (feat-major epilogue): rod[n] = ri[n]*od[n], replicated 128 part
    rod = (ri * od).astype(np.float32)
    rodT = np.ones((NC_CORES, 128, npad), np.float32)
    for c in range(NC_CORES):
        vv = np.ones(npad, np.float32)
        vv[:npc] = rod[c * npc:(c + 1) * npc]
        rodT[c] = np.tile(vv[None, :], (128, 1))

    W0p4 = np.zeros((4, D), np.float32)
    W0p4[:3] = np.asarray(W0, np.float32)
    W0p4[3] = np.asarray(b0, np.float32)

    consts = {
        "iota_bc": np.tile(np.arange(128, dtype=np.float32)[None, :], (128, 1)),
        "b1_bc": np.tile(np.asarray(b1, np.float32)[None, :], (128, 1)),
        "W0p4": W0p4,
        "W1": np.asarray(W1, np.float32),
    }
    return dict(
        n=n, e=e, npc=npc, n_chunks=n_chunks, npad=npad, half=half,
        t0k=t0k, x0off=x0off, tot0=tot0,
        tkb=tkb, dvoff=dvoff, tot_tiles=tot_tiles, n_groups=n_groups,
        gslices=gslices, Tmax=Tmax,
        od=od, ri=ri, in_deg=in_deg, ri_pc=_pc(ri), rodT=rodT, vcnt=vcnt,
        x0h=x0h, dv0=dv0, dv1=dv1, idxh=idxh, consts=consts,
        b0=np.asarray(b0, np.float32), W1f=np.asarray(W1, np.float32),
    )


# --------------------------------------------------------------------------
# device programs
# --------------------------------------------------------------------------

def _new_nc(nq=1):
    return bacc.Bacc("TRN2", target_bir_lowering=False, debug=False,
                     num_devices=NC_CORES, num_swdge_queues=nq)


@with_exitstack
def _conv0_body(ctx: ExitStack, tc, aps, n_chunks, t0k, x0off, tot0):
    nc = tc.nc
    cpool = ctx.enter_context(tc.tile_pool(name="consts", bufs=1))
    pool = ctx.enter_context(tc.tile_pool(name="work", bufs=3))
    epool = ctx.enter_context(tc.tile_pool(name="epi", bufs=4))
    ps_a = ctx.enter_context(tc.tile_pool(name="ps_a", bufs=2, space="PSUM"))
    ps_g = ctx.enter_context(tc.tile_pool(name="ps_g", bufs=3, space="PSUM"))
    ps_q = ctx.enter_context(tc.tile_pool(name="ps_q", bufs=2, space="PSUM"))

    iota_sb = cpool.tile([128, 128], BF16)
    nc.sync.dma_start(iota_sb[:], aps["iota_bc"][:])
    w0_sb = cpool.tile([4, D], BF16)
    nc.sync.dma_start(w0_sb[:], aps["W0p4"][:])
    w1_sb = cpool.tile([D, D], BF16)
    nc.sync.dma_start(w1_sb[:], aps["W1"][:])
    dv0_sb = cpool.tile([128, tot0], BF16)
    for j in range(4):
        a, bnd = (tot0 * j) // 4, (tot0 * (j + 1)) // 4
        nc.sync.dma_start(dv0_sb[:, a:bnd], aps["dv0"][:, a:bnd])
    rod_sb = cpool.tile([128, n_chunks * 128], F32)
    for j in range(8):
        a = (n_chunks * j) // 8 * 128
        bnd = (n_chunks * (j + 1)) // 8 * 128
        nc.sync.dma_start(rod_sb[:, a:bnd], aps["rodT"][:, a:bnd])
    x0_sb = cpool.tile([128, tot0 * 4], BF16)
    for j in range(8):
        a, bnd = (tot0 * j) // 8 * 4, (tot0 * (j + 1)) // 8 * 4
        nc.sync.dma_start(x0_sb[:, a:bnd], aps["x0h"][:, a:bnd])
    hq_d = aps["hq"]        # [n_chunks*128, D] bf16 node-major output

    t0max = int(max(int(t) for t in t0k))
    for k in range(n_chunks):
        ntk = int(t0k[k])
        o0 = int(x0off[k])
        x0_k = x0_sb[:, o0 * 4:(o0 + ntk) * 4]
        ind_sb = pool.tile([128, t0max * 128], BF16, tag="ind")
        nc.vector.tensor_tensor(
            ind_sb[:, :ntk * 128].rearrange("p (t j) -> p t j", j=128),
            dv0_sb[:, o0:o0 + ntk].unsqueeze(2).broadcast_to([128, ntk, 128]),
            iota_sb[:].unsqueeze(1).broadcast_to([128, ntk, 128]),
            AluOpType.is_equal,
        )
        agg_ps = ps_a.tile([4, 128], F32, tag="agg")
        for t in range(ntk):
            nc.tensor.matmul(
                agg_ps[:],
                lhsT=x0_k[:, bass.ts(t, 4)],
                rhs=ind_sb[:, bass.ts(t, 128)],
                start=(t == 0),
                stop=(t == ntk - 1),
            )
        agg_sb = epool.tile([4, 128], BF16, tag="aggsb")
        nc.scalar.activation(agg_sb[:], agg_ps[:],
                             mybir.ActivationFunctionType.Copy)

        # gT[f, n] = W0p4^T @ agg  (+ b0 folded via payload col3)
        g_ps = ps_g.tile([128, 128], F32, tag="g")
        nc.tensor.matmul(g_ps[:], lhsT=w0_sb[:], rhs=agg_sb[:],
                         start=True, stop=True)

        # v = leaky(gT * rod)   (od folded: od*leaky(x) == leaky(od*x))
        z_sb = epool.tile([128, 128], F32, tag="z")
        nc.vector.tensor_tensor(z_sb[:], g_ps[:],
                                rod_sb[:, k * 128:(k + 1) * 128],
                                AluOpType.mult)
        v_sb = epool.tile([128, 128], BF16, tag="v")
        nc.vector.scalar_tensor_tensor(
            v_sb[:], z_sb[:], float(NEG_SLOPE), z_sb[:],
            AluOpType.mult, AluOpType.max,
        )

        # hq[n, f'] = v^T @ W1  (node-major)
        q_ps = ps_q.tile([128, D], F32, tag="q")
        nc.tensor.matmul(q_ps[:], lhsT=v_sb[:], rhs=w1_sb[:],
                         start=True, stop=True)
        hq_sb = epool.tile([128, D], BF16, tag="hq")
        nc.scalar.activation(hq_sb[:], q_ps[:],
                             mybir.ActivationFunctionType.Copy)
        nc.sync.dma_start(hq_d[k * 128:(k + 1) * 128, :], hq_sb[:])


@with_exitstack
def _conv1_body(ctx: ExitStack, tc, aps, p):
    nc = tc.nc
    n, half = p["n"], p["half"]
    n_chunks, tkb, dvoff = p["n_chunks"], p["tkb"], p["dvoff"]
    n_groups, gslices, Tmax = p["n_groups"], p["gslices"], p["Tmax"]

    cpool = ctx.enter_context(tc.tile_pool(name="consts", bufs=1))
    xpool = ctx.enter_context(tc.tile_pool(name="x", bufs=6))
    ipool = ctx.enter_context(tc.tile_pool(name="ind", bufs=3))
    epool = ctx.enter_context(tc.tile_pool(name="epi", bufs=4))
    ps_o = ctx.enter_context(tc.tile_pool(name="ps_o", bufs=4, space="PSUM"))

    iota_sb = cpool.tile([128, 128], BF16)
    nc.sync.dma_start(iota_sb[:], aps["iota_bc"][:])
    b1_sb = cpool.tile([128, 128], F32)
    nc.sync.dma_start(b1_sb[:], aps["b1_bc"][:])
    ri_sb = cpool.tile([128, n_chunks], F32)
    nc.sync.dma_start(ri_sb[:], aps["ri_pc"][:])
    dv1_sb = cpool.tile([128, p["tot_tiles"]], BF16)
    nc.sync.dma_start(dv1_sb[:], aps["dv1"][:])
    idx_sb = cpool.tile([128, 2 * n_groups * Tmax * 8], I16)
    nc.sync.dma_start(idx_sb[:], aps["idxh"][:])


    hq_d = aps["hq"]        # [n, D] bf16
    out_d = aps["out"]      # [n_chunks*128, D] f32

    qrr = 0
    for g in range(n_groups):
        ks = gslices[g]
        xb = []
        for b in range(2):
            Tg = int(sum(int(tkb[k, b]) for k in ks))
            x_sb = xpool.tile([128, Tmax * D], BF16, tag=f"x{b}")
            src_rows = hq_d[0:half, :] if b == 0 else hq_d[half:n, :]
            nidx = Tg * 128
            nc.gpsimd.dma_gather(
                out_ap=x_sb[:, :Tg * D].rearrange("p (t f) -> p t f", f=D),
                in_ap=src_rows,
                idxs_ap=idx_sb[:, (b * n_groups + g) * Tmax * 8:
                               (b * n_groups + g) * Tmax * 8 + Tg * 8],
                num_idxs=nidx,
                num_idxs_reg=nidx,
                elem_size=D,
                single_packet=(nidx // 16 <= 63),
                queue_num=qrr % NQUEUES,
            )
            qrr += 1
            xb.append(x_sb)

        for k in ks:
            nt_a, nt_b = int(tkb[k, 0]), int(tkb[k, 1])
            o_a, o_b = int(dvoff[k, 0]), int(dvoff[k, 1])
            nt = nt_a + nt_b
            ind_sb = ipool.tile([128, nt * 128], BF16, tag="ind")
            # indicator for bucket-a tiles then bucket-b tiles of this chunk
            nc.vector.tensor_tensor(
                ind_sb[:, :nt_a * 128].rearrange("p (t j) -> p t j", j=128),
                dv1_sb[:, o_a:o_a + nt_a].unsqueeze(2)
                    .broadcast_to([128, nt_a, 128]),
                iota_sb[:].unsqueeze(1).broadcast_to([128, nt_a, 128]),
                AluOpType.is_equal,
            )
            nc.vector.tensor_tensor(
                ind_sb[:, nt_a * 128:].rearrange("p (t j) -> p t j", j=128),
                dv1_sb[:, o_b:o_b + nt_b].unsqueeze(2)
                    .broadcast_to([128, nt_b, 128]),
                iota_sb[:].unsqueeze(1).broadcast_to([128, nt_b, 128]),
                AluOpType.is_equal,
            )
            # node-major segment-sum: o_ps[128 dst, D] accumulates
            o_ps = ps_o.tile([128, D], F32, tag="o")
            # tile offsets of this chunk within the group's x buffers
            ta0 = int(sum(int(tkb[kk, 0]) for kk in ks if kk < k))
            tb0 = int(sum(int(tkb[kk, 1]) for kk in ks if kk < k))
            mm = 0
            for t in range(nt_a):
                nc.tensor.matmul(
                    o_ps[:],
                    lhsT=ind_sb[:, bass.ts(t, 128)],
                    rhs=xb[0][:, bass.ts(ta0 + t, D)],
                    start=(mm == 0), stop=(mm == nt - 1),
                )
                mm += 1
            for t in range(nt_b):
                nc.tensor.matmul(
                    o_ps[:],
                    lhsT=ind_sb[:, bass.ts(nt_a + t, 128)],
                    rhs=xb[1][:, bass.ts(tb0 + t, D)],
                    start=(mm == 0), stop=(mm == nt - 1),
                )
                mm += 1

            out_sb = epool.tile([128, D], F32, tag="outsb")
            nc.vector.scalar_tensor_tensor(
                out_sb[:], o_ps[:], ri_sb[:, k:k + 1], b1_sb[:],
                AluOpType.mult, AluOpType.add,
            )
            nc.sync.dma_start(out_d[k * 128:(k + 1) * 128, :], out_sb[:])


def tensor_specs0(p):
    n_chunks, tot0 = p["n_chunks"], p["tot0"]
    return {
        "iota_bc": ((128, 128), BF16, "ExternalInput"),
        "W0p4": ((4, D), BF16, "ExternalInput"),
        "W1": ((D, D), BF16, "ExternalInput"),
        "dv0": ((128, tot0), BF16, "ExternalInput"),
        "rodT": ((128, n_chunks * 128), F32, "ExternalInput"),
        "x0h": ((128, tot0 * 4), BF16, "ExternalInput"),
        "hq": ((n_chunks * 128, D), BF16, "ExternalOutput"),
    }


def tensor_specs1(p):
    n, n_chunks = p["n"], p["n_chunks"]
    return {
        "iota_bc": ((128, 128), BF16, "ExternalInput"),
        "b1_bc": ((128, 128), F32, "ExternalInput"),
        "ri_pc": ((128, n_chunks), F32, "ExternalInput"),
        "dv1": ((128, p["tot_tiles"]), BF16, "ExternalInput"),
        "idxh": ((128, 2 * p["n_groups"] * p["Tmax"] * 8), I16, "ExternalInput"),
        "hq": ((n, D), BF16, "ExternalInput"),
        "out": ((n_chunks * 128, D), F32, "ExternalOutput"),
    }


def in_maps0(p):
    c = p["consts"]
    return [
        {"iota_bc": c["iota_bc"], "W0p4": c["W0p4"], "W1": c["W1"],
         "dv0": p["dv0"][i], "rodT": p["rodT"][i], "x0h": p["x0h"][i]}
        for i in range(NC_CORES)
    ]


def in_maps1(p, hq_full):
    c = p["consts"]
    return [
        {"iota_bc": c["iota_bc"], "b1_bc": c["b1_bc"], "ri_pc": p["ri_pc"][i],
         "dv1": p["dv1"][i],
         "idxh": np.ascontiguousarray(
             p["idxh"][i].reshape(2 * p["n_groups"], 128, p["Tmax"] * 8)
             .transpose(1, 0, 2).reshape(128, -1)),
         "hq": hq_full}
        for i in range(NC_CORES)
    ]


def _build(body, tensors, nq=1, **kw):
    nc = _new_nc(nq)
    aps = {
        name: nc.dram_tensor(name, list(shape), dtype, kind=kind).ap()
        for name, (shape, dtype, kind) in tensors.items()
    }
    with tile.TileContext(nc) as tc:
        body(tc, aps, **kw)
    nc.compile()
    return nc


class _SimResults:
    def __init__(self, results):
        self.results = results
        self.exec_time_ns = None


def _run_sim(nc, in_maps, out_names):
    from concourse.bass_interp import CoreSim
    results = []
    for im in in_maps:
        sim = CoreSim(nc)
        for k, v in im.items():
            sim.tensor(k)[:] = v
        sim.simulate(check_with_hw=False)
        results.append({k: np.array(sim.tensor(k)) for k in out_names})
    return _SimResults(results)


# --------------------------------------------------------------------------
# entry point
# --------------------------------------------------------------------------

def _to_bf16(a):
    import ml_dtypes
    return np.asarray(a).astype(ml_dtypes.bfloat16)


def kernel(src, dst, weight, significance, emb, W0, b0, W1, b1):
    global LAST_EXEC_TIMES_NS, LAST_RESULTS
    LAST_EXEC_TIMES_NS = []
    LAST_RESULTS = []
    trace = bool(os.environ.get("BASS_TRACE"))

    p = _prep(src, dst, weight, significance, emb, W0, b0, W1, b1)
    n, npc, n_chunks = p["n"], p["npc"], p["n_chunks"]
    c = p["consts"]

    # bf16-cast inputs where specs say BF16
    c["iota_bc"] = _to_bf16(c["iota_bc"])
    c["W0p4"] = _to_bf16(c["W0p4"])
    c["W1"] = _to_bf16(c["W1"])
    p["dv0"] = _to_bf16(p["dv0"])
    p["x0h"] = _to_bf16(p["x0h"])
    p["dv1"] = _to_bf16(p["dv1"])

    use_sim = bool(os.environ.get("BASS_SIM"))

    nc0 = _build(_conv0_body, tensor_specs0(p), n_chunks=n_chunks,
                 t0k=p["t0k"], x0off=p["x0off"], tot0=p["tot0"])
    if use_sim:
        res0 = _run_sim(nc0, in_maps0(p), ["hq"])
    else:
        res0 = run_bass_kernel_spmd(nc0, in_maps0(p),
                                    core_ids=list(range(NC_CORES)), trace=trace)
    LAST_RESULTS.append(res0)
    LAST_EXEC_TIMES_NS.append(res0.exec_time_ns)
    hq_full = np.concatenate(
        [np.asarray(res0.results[i]["hq"][:npc]) for i in range(NC_CORES)], axis=0
    )
    assert hq_full.shape == (n, D)

    # patch zero-in-degree nodes (b0 fold drops them): hq = leaky(b0)*od @ W1
    zid = np.nonzero(p["in_deg"] == 0)[0]
    if zid.size:
        b0f = p["b0"]
        v = np.where(b0f > 0, b0f, NEG_SLOPE * b0f)[None, :] * p["od"][zid][:, None]
        hq_full[zid] = _to_bf16(v @ p["W1f"])

    nc1 = _build(_conv1_body, tensor_specs1(p), nq=NQUEUES, p=p)
    if use_sim:
        res1 = _run_sim(nc1, in_maps1(p, hq_full), ["out"])
    else:
        res1 = run_bass_kernel_spmd(nc1, in_maps1(p, hq_full),
                                    core_ids=list(range(NC_CORES)), trace=trace)
    LAST_RESULTS.append(res1)
    LAST_EXEC_TIMES_NS.append(res1.exec_time_ns)

    out = np.concatenate(
        [res1.results[i]["out"][:npc] for i in range(NC_CORES)], axis=0
    )
    assert out.shape == (n, D)
    return out.astype(np.float32)


# revision 33
# speedup vs baseline: 1.1999x; 1.0073x over previous
"""GCN (2-layer GraphConv, norm='both') on 8 Trainium2 NeuronCores.

Strategy (node-partition / in-edge pull, 2 dispatches):
  Dispatch 0 (conv0 + W1 fusion, bf16 matmuls):
    - Nodes in 8 contiguous shards; edges grouped by dst, chunked per 128 dsts.
    - Per chunk: indicator matmuls aggregate the host-laid-out 4-wide edge
      payload (3 od-scaled features + a 1/(ri*indeg) column that folds b0 in),
      then gT = W0p^T @ agg gives feat-major conv0 output, epilogue applies
      rod = ri*od and leaky (od folded inside: od*leaky(x) = leaky(od*x)),
      and hq = v @ W1 is computed per NODE (linearity: segsum(v[src]) @ W1 ==
      segsum((v@W1)[src])), written node-major in bf16.
  Dispatch 1 (conv1 = pure segment-sum of hq rows):
    - dma_gather (SWDGE) pulls hq[src] rows (256B bf16) for merged groups of
      chunks per call, per-chunk tile counts exact (trailing -1 idx are
      skipped by the Q7 ucode; only intra-chunk pad-to-128 costs descriptors).
    - Node-major indicator matmuls (lhsT=ind, rhs=x) segment-sum directly
      into [128 dst, D] PSUM; epilogue applies ri and b1.

All O(E*D)/O(N*D) compute and memory traffic runs on-device; the host does
index manipulation (sort/pad/relabel), normalization constants, and the
4-float-per-edge conv0 payload layout.
"""

import os
from contextlib import ExitStack

import numpy as np

import concourse.bass as bass
import concourse.tile as tile
from concourse import bacc, mybir
from concourse._compat import with_exitstack
from concourse.alu_op_type import AluOpType
from concourse.bass_utils import run_bass_kernel_spmd

F32 = mybir.dt.float32
BF16 = mybir.dt.bfloat16
I16 = mybir.dt.int16

NC_CORES = 8
D = 128
NEG_SLOPE = 0.01
GCHUNK = 1        # dst-chunks merged per dma_gather call
NQUEUES = 4       # SWDGE queues (disjoint Q7 cpu pairs -> parallel desc-gen)

LAST_EXEC_TIMES_NS: list = []
LAST_RESULTS: list = []


# --------------------------------------------------------------------------
# host-side prep
# --------------------------------------------------------------------------

def _wrap_idx(idx: np.ndarray) -> np.ndarray:
    """dma_gather index layout: position i lives at [i % 16, i // 16] of a
    16-row wrap, replicated 8x (one copy per Q7 core) -> [128, n/16] int16."""
    n = idx.shape[0]
    assert n % 16 == 0
    return np.tile(idx.astype(np.int16).reshape(n // 16, 16).T, (8, 1))


def _prep(src, dst, weight, significance, emb, W0, b0, W1, b1):
    n = weight.shape[0]
    e = src.shape[0]
    npc = n // NC_CORES
    assert npc * NC_CORES == n
    n_chunks = (npc + 127) // 128
    npad = n_chunks * 128
    half = (n + 1) // 2
    assert half <= 32767 and n - half <= 32767

    src = np.asarray(src).astype(np.int64)
    dst = np.asarray(dst).astype(np.int64)

    out_deg = np.bincount(src, minlength=n).astype(np.float64)
    in_deg = np.bincount(dst, minlength=n).astype(np.float64)
    od = (1.0 / np.sqrt(np.clip(out_deg, 1.0, None))).astype(np.float32)
    ri = (1.0 / np.sqrt(np.clip(in_deg, 1.0, None))).astype(np.float32)
    # b0 folding: payload col3 sums to 1/ri over each dst's in-edges
    with np.errstate(divide="ignore"):
        c_dst = np.where(in_deg > 0, 1.0 / (ri * np.maximum(in_deg, 1.0)), 0.0)
    c_dst = c_dst.astype(np.float32)

    emb_rows = np.asarray(emb, np.float32)[np.asarray(significance).astype(np.int64)]
    feats = np.concatenate(
        [np.asarray(weight, np.float32)[:, None], emb_rows], axis=1
    ) * od[:, None]                                        # [n, 3]

    order = np.argsort(dst, kind="stable")
    s_src, s_dst = src[order], dst[order]

    core_of = s_dst // npc
    loc = s_dst - core_of * npc
    chunk_of = loc // 128
    e_starts = np.searchsorted(core_of * n_chunks + chunk_of,
                               np.arange(NC_CORES * n_chunks + 1))

    # ---- conv0: per-chunk exact tile counts (max over cores) ----
    cnt_all = np.diff(e_starts).reshape(NC_CORES, n_chunks)
    t0k = np.maximum(1, -(-cnt_all.max(axis=0) // 128)).astype(np.int64)
    x0off = np.zeros(n_chunks + 1, np.int64)
    np.cumsum(t0k, out=x0off[1:])
    tot0 = int(x0off[-1])

    x0h = np.zeros((NC_CORES, 128, tot0 * 4), np.float32)
    dv0 = np.full((NC_CORES, 128, tot0), -1.0, np.float32)

    # ---- conv1: per (chunk,bucket) exact tile counts, shared across cores ----
    m0 = (s_src < half)
    # per (core, chunk, bucket) edge counts
    cnt_b = np.zeros((NC_CORES, n_chunks, 2), np.int64)
    for c in range(NC_CORES):
        for k in range(n_chunks):
            s0, s1 = e_starts[c * n_chunks + k], e_starts[c * n_chunks + k + 1]
            lo = int(np.count_nonzero(m0[s0:s1]))
            cnt_b[c, k, 0], cnt_b[c, k, 1] = lo, (s1 - s0) - lo
    # shared per-(chunk,bucket) tile count = max over cores
    tkb = np.maximum(1, -(-cnt_b.max(axis=0) // 128))      # [n_chunks, 2]

    n_groups = -(-n_chunks // GCHUNK)
    # per (group,bucket): total tiles, shared
    gslices = [list(range(g * GCHUNK, min((g + 1) * GCHUNK, n_chunks)))
               for g in range(n_groups)]
    Tg = np.array([[int(tkb[ks, b].sum()) for b in range(2)] for ks in gslices])
    Tmax = int(Tg.max())

    # host buffers for conv1 idx / dv (idx unwrapped, length Tmax*128/call)
    idx_w = np.zeros((NC_CORES, 2, n_groups, Tmax * 128), np.int64)
    # dv1 layout: for chunk k bucket b, tiles occupy columns
    # [dvoff[k,b], dvoff[k,b]+tkb[k,b]) each of width 128 (flattened later)
    dvoff = np.zeros((n_chunks, 2), np.int64)
    acc = 0
    for k in range(n_chunks):
        for b in range(2):
            dvoff[k, b] = acc
            acc += int(tkb[k, b])
    tot_tiles = acc

    dv1 = np.full((NC_CORES, 128, tot_tiles), -1.0, np.float32)

    for c in range(NC_CORES):
        for k in range(n_chunks):
            s0, s1 = e_starts[c * n_chunks + k], e_starts[c * n_chunks + k + 1]
            es, ed = s_src[s0:s1], s_dst[s0:s1]
            dloc = (ed - c * npc - k * 128).astype(np.float32)

            # conv0 payload (col3 = c_dst of the edge's dst)
            ntk = int(t0k[k])
            n0k = ntk * 128
            o0 = int(x0off[k])
            pay = np.zeros((n0k, 4), np.float32)
            nreal = s1 - s0
            pay[:nreal, :3] = feats[es]
            pay[:nreal, 3] = c_dst[ed]
            x0h[c, :, o0 * 4:(o0 + ntk) * 4] = (
                pay.reshape(ntk, 128, 4).transpose(1, 0, 2).reshape(128, ntk * 4)
            )
            dvc = np.full(n0k, -1.0, np.float32)
            dvc[:nreal] = dloc
            dv0[c, :, o0:o0 + ntk] = dvc.reshape(ntk, 128).T

            # conv1 per bucket
            msk = m0[s0:s1]
            for b in range(2):
                sel = msk if b == 0 else ~msk
                sb = es[sel] - (0 if b == 0 else half)
                db = dloc[sel]
                nt = int(tkb[k, b])
                nb = nt * 128
                g = k // GCHUNK
                # pad with valid index 0 (indicator is -1 there, so the
                # gathered row contributes nothing)
                pidx = np.zeros(nb, np.int64)
                pidx[:sb.shape[0]] = sb
                dvb = np.full(nb, -1.0, np.float32)
                dvb[:db.shape[0]] = db
                o = dvoff[k, b]
                dv1[c, :, o:o + nt] = dvb.reshape(nt, 128).T
                toff = int(sum(int(tkb[kk, b]) for kk in gslices[g] if kk < k))
                idx_w[c, b, g, toff * 128:toff * 128 + nb] = pidx

    # wrap conv1 idx: per (bucket, group) one gather call of Tg[g,b]*128 idx,
    # padded (with trailing -1) to Tmax*128 so shapes are shared
    idxh = np.zeros((NC_CORES, 2, n_groups, 128, Tmax * 8), np.int16)
    for c in range(NC_CORES):
        for b in range(2):
            for g in range(n_groups):
                idxh[c, b, g] = _wrap_idx(idx_w[c, b, g])

    vcnt = np.zeros((NC_CORES, 2 * n_groups), np.int32)
    for c in range(NC_CORES):
        for b in range(2):
            for g in range(n_groups):
                vcnt[c, b * n_groups + g] = int(
                    sum(int(cnt_b[c, kk, b]) for kk in gslices[g]))

    def _pc(v):
        out = np.ones((NC_CORES, 128, n_chunks), np.float32)
        for c in range(NC_CORES):
            vv = np.ones(npad, np.float32)
            vv[:npc] = v[c * npc:(c + 1) * npc]
            out[c] = vv.reshape(n_chunks, 128).T
        return out

    # rod row (feat-major epilogue): rod[n] = ri[n]*od[n], replicated 128 part
    rod = (ri * od).astype(np.float32)
    rodT = np.ones((NC_CORES, 128, npad), np.float32)
    for c in range(NC_CORES):
        vv = np.ones(npad, np.float32)
        vv[:npc] = rod[c * npc:(c + 1) * npc]
        rodT[c] = np.tile(vv[None, :], (128, 1))

    W0p4 = np.zeros((4, D), np.float32)
    W0p4[:3] = np.asarray(W0, np.float32)
    W0p4[3] = np.asarray(b0, np.float32)

    consts = {
        "iota_bc": np.tile(np.arange(128, dtype=np.float32)[None, :], (128, 1)),
        "b1_bc": np.tile(np.asarray(b1, np.float32)[None, :], (128, 1)),
        "W0p4": W0p4,
        "W1": np.asarray(W1, np.float32),
    }
    return dict(
        n=n, e=e, npc=npc, n_chunks=n_chunks, npad=npad, half=half,
        t0k=t0k, x0off=x0off, tot0=tot0,
        tkb=tkb, dvoff=dvoff, tot_tiles=tot_tiles, n_groups=n_groups,
        gslices=gslices, Tmax=Tmax,
        od=od, ri=ri, in_deg=in_deg, ri_pc=_pc(ri), rodT=rodT, vcnt=vcnt,
        x0h=x0h, dv0=dv0, dv1=dv1, idxh=idxh, consts=consts,
        b0=np.asarray(b0, np.float32), W1f=np.asarray(W1, np.float32),
    )


# --------------------------------------------------------------------------
# device programs
# --------------------------------------------------------------------------

def _new_nc(nq=1):
    return bacc.Bacc("TRN2", target_bir_lowering=False, debug=False,
                     num_devices=NC_CORES, num_swdge_queues=nq)


@with_exitstack
def _conv0_body(ctx: ExitStack, tc, aps, n_chunks, t0k, x0off, tot0):
    nc = tc.nc
    cpool = ctx.enter_context(tc.tile_pool(name="consts", bufs=1))
    pool = ctx.enter_context(tc.tile_pool(name="work", bufs=3))
    epool = ctx.enter_context(tc.tile_pool(name="epi", bufs=4))
    ps_a = ctx.enter_context(tc.tile_pool(name="ps_a", bufs=2, space="PSUM"))
    ps_g = ctx.enter_context(tc.tile_pool(name="ps_g", bufs=3, space="PSUM"))
    ps_q = ctx.enter_context(tc.tile_pool(name="ps_q", bufs=2, space="PSUM"))

    iota_sb = cpool.tile([128, 128], BF16)
    nc.sync.dma_start(iota_sb[:], aps["iota_bc"][:])
    w0_sb = cpool.tile([4, D], BF16)
    nc.sync.dma_start(w0_sb[:], aps["W0p4"][:])
    w1_sb = cpool.tile([D, D], BF16)
    nc.sync.dma_start(w1_sb[:], aps["W1"][:])
    dv0_sb = cpool.tile([128, tot0], BF16)
    for j in range(4):
        a, bnd = (tot0 * j) // 4, (tot0 * (j + 1)) // 4
        nc.sync.dma_start(dv0_sb[:, a:bnd], aps["dv0"][:, a:bnd])
    rod_sb = cpool.tile([128, n_chunks * 128], F32)
    for j in range(8):
        a = (n_chunks * j) // 8 * 128
        bnd = (n_chunks * (j + 1)) // 8 * 128
        nc.sync.dma_start(rod_sb[:, a:bnd], aps["rodT"][:, a:bnd])
    x0_sb = cpool.tile([128, tot0 * 4], BF16)
    for j in range(8):
        a, bnd = (tot0 * j) // 8 * 4, (tot0 * (j + 1)) // 8 * 4
        nc.sync.dma_start(x0_sb[:, a:bnd], aps["x0h"][:, a:bnd])
    hq_d = aps["hq"]        # [n_chunks*128, D] bf16 node-major output

    t0max = int(max(int(t) for t in t0k))
    for k in range(n_chunks):
        ntk = int(t0k[k])
        o0 = int(x0off[k])
        x0_k = x0_sb[:, o0 * 4:(o0 + ntk) * 4]
        ind_sb = pool.tile([128, t0max * 128], BF16, tag="ind")
        nc.vector.tensor_tensor(
            ind_sb[:, :ntk * 128].rearrange("p (t j) -> p t j", j=128),
            dv0_sb[:, o0:o0 + ntk].unsqueeze(2).broadcast_to([128, ntk, 128]),
            iota_sb[:].unsqueeze(1).broadcast_to([128, ntk, 128]),
            AluOpType.is_equal,
        )
        agg_ps = ps_a.tile([4, 128], F32, tag="agg")
        for t in range(ntk):
            nc.tensor.matmul(
                agg_ps[:],
                lhsT=x0_k[:, bass.ts(t, 4)],
                rhs=ind_sb[:, bass.ts(t, 128)],
                start=(t == 0),
                stop=(t == ntk - 1),
            )
        agg_sb = epool.tile([4, 128], BF16, tag="aggsb")
        nc.scalar.activation(agg_sb[:], agg_ps[:],
                             mybir.ActivationFunctionType.Copy)

        # gT[f, n] = W0p4^T @ agg  (+ b0 folded via payload col3)
        g_ps = ps_g.tile([128, 128], F32, tag="g")
        nc.tensor.matmul(g_ps[:], lhsT=w0_sb[:], rhs=agg_sb[:],
                         start=True, stop=True)

        # v = leaky(gT * rod)   (od folded: od*leaky(x) == leaky(od*x))
        z_sb = epool.tile([128, 128], F32, tag="z")
        nc.vector.tensor_tensor(z_sb[:], g_ps[:],
                                rod_sb[:, k * 128:(k + 1) * 128],
                                AluOpType.mult)
        v_sb = epool.tile([128, 128], BF16, tag="v")
        nc.vector.scalar_tensor_tensor(
            v_sb[:], z_sb[:], float(NEG_SLOPE), z_sb[:],
            AluOpType.mult, AluOpType.max,
        )

        # hq[n, f'] = v^T @ W1  (node-major)
        q_ps = ps_q.tile([128, D], F32, tag="q")
        nc.tensor.matmul(q_ps[:], lhsT=v_sb[:], rhs=w1_sb[:],
                         start=True, stop=True)
        hq_sb = epool.tile([128, D], BF16, tag="hq")
        nc.scalar.activation(hq_sb[:], q_ps[:],
                             mybir.ActivationFunctionType.Copy)
        nc.sync.dma_start(hq_d[k * 128:(k + 1) * 128, :], hq_sb[:])


@with_exitstack
def _conv1_body(ctx: ExitStack, tc, aps, p):
    nc = tc.nc
    n, half = p["n"], p["half"]
    n_chunks, tkb, dvoff = p["n_chunks"], p["tkb"], p["dvoff"]
    n_groups, gslices, Tmax = p["n_groups"], p["gslices"], p["Tmax"]

    cpool = ctx.enter_context(tc.tile_pool(name="consts", bufs=1))
    xpool = ctx.enter_context(tc.tile_pool(name="x", bufs=6))
    ipool = ctx.enter_context(tc.tile_pool(name="ind", bufs=3))
    epool = ctx.enter_context(tc.tile_pool(name="epi", bufs=4))
    ps_o = ctx.enter_context(tc.tile_pool(name="ps_o", bufs=4, space="PSUM"))

    iota_sb = cpool.tile([128, 128], BF16)
    nc.sync.dma_start(iota_sb[:], aps["iota_bc"][:])
    b1_sb = cpool.tile([128, 128], F32)
    nc.sync.dma_start(b1_sb[:], aps["b1_bc"][:])
    ri_sb = cpool.tile([128, n_chunks], F32)
    nc.sync.dma_start(ri_sb[:], aps["ri_pc"][:])
    dv1_sb = cpool.tile([128, p["tot_tiles"]], BF16)
    nc.sync.dma_start(dv1_sb[:], aps["dv1"][:])
    idx_sb = cpool.tile([128, 2 * n_groups * Tmax * 8], I16)
    nc.sync.dma_start(idx_sb[:], aps["idxh"][:])


    hq_d = aps["hq"]        # [n, D] bf16
    out_d = aps["out"]      # [n_chunks*128, D] f32

    qrr = 0
    for g in range(n_groups):
        ks = gslices[g]
        xb = []
        for b in range(2):
            Tg = int(sum(int(tkb[k, b]) for k in ks))
            x_sb = xpool.tile([128, Tmax * D], BF16, tag=f"x{b}")
            src_rows = hq_d[0:half, :] if b == 0 else hq_d[half:n, :]
            nidx = Tg * 128
            nc.gpsimd.dma_gather(
                out_ap=x_sb[:, :Tg * D].rearrange("p (t f) -> p t f", f=D),
                in_ap=src_rows,
                idxs_ap=idx_sb[:, (b * n_groups + g) * Tmax * 8:
                               (b * n_groups + g) * Tmax * 8 + Tg * 8],
                num_idxs=nidx,
                num_idxs_reg=nidx,
                elem_size=D,
                single_packet=(nidx // 16 <= 63),
                queue_num=qrr % NQUEUES,
            )
            qrr += 1
            xb.append(x_sb)

        for k in ks:
            nt_a, nt_b = int(tkb[k, 0]), int(tkb[k, 1])
            o_a, o_b = int(dvoff[k, 0]), int(dvoff[k, 1])
            nt = nt_a + nt_b
            ind_sb = ipool.tile([128, nt * 128], BF16, tag="ind")
            # indicator for bucket-a tiles then bucket-b tiles of this chunk
            nc.vector.tensor_tensor(
                ind_sb[:, :nt_a * 128].rearrange("p (t j) -> p t j", j=128),
                dv1_sb[:, o_a:o_a + nt_a].unsqueeze(2)
                    .broadcast_to([128, nt_a, 128]),
                iota_sb[:].unsqueeze(1).broadcast_to([128, nt_a, 128]),
                AluOpType.is_equal,
            )
            nc.vector.tensor_tensor(
                ind_sb[:, nt_a * 128:].rearrange("p (t j) -> p t j", j=128),
                dv1_sb[:, o_b:o_b + nt_b].unsqueeze(2)
                    .broadcast_to([128, nt_b, 128]),
                iota_sb[:].unsqueeze(1).broadcast_to([128, nt_b, 128]),
                AluOpType.is_equal,
            )
            # node-major segment-sum: o_ps[128 dst, D] accumulates
            o_ps = ps_o.tile([128, D], F32, tag="o")
            # tile offsets of this chunk within the group's x buffers
            ta0 = int(sum(int(tkb[kk, 0]) for kk in ks if kk < k))
            tb0 = int(sum(int(tkb[kk, 1]) for kk in ks if kk < k))
            mm = 0
            for t in range(nt_a):
                nc.tensor.matmul(
                    o_ps[:],
                    lhsT=ind_sb[:, bass.ts(t, 128)],
                    rhs=xb[0][:, bass.ts(ta0 + t, D)],
                    start=(mm == 0), stop=(mm == nt - 1),
                )
                mm += 1
            for t in range(nt_b):
                nc.tensor.matmul(
                    o_ps[:],
                    lhsT=ind_sb[:, bass.ts(nt_a + t, 128)],
                    rhs=xb[1][:, bass.ts(tb0 + t, D)],
                    start=(mm == 0), stop=(mm == nt - 1),
                )
                mm += 1

            out_sb = epool.tile([128, D], F32, tag="outsb")
            nc.vector.scalar_tensor_tensor(
                out_sb[:], o_ps[:], ri_sb[:, k:k + 1], b1_sb[:],
                AluOpType.mult, AluOpType.add,
            )
            nc.sync.dma_start(out_d[k * 128:(k + 1) * 128, :], out_sb[:])


def tensor_specs0(p):
    n_chunks, tot0 = p["n_chunks"], p["tot0"]
    return {
        "iota_bc": ((128, 128), BF16, "ExternalInput"),
        "W0p4": ((4, D), BF16, "ExternalInput"),
        "W1": ((D, D), BF16, "ExternalInput"),
        "dv0": ((128, tot0), BF16, "ExternalInput"),
        "rodT": ((128, n_chunks * 128), F32, "ExternalInput"),
        "x0h": ((128, tot0 * 4), BF16, "ExternalInput"),
        "hq": ((n_chunks * 128, D), BF16, "ExternalOutput"),
    }


def tensor_specs1(p):
    n, n_chunks = p["n"], p["n_chunks"]
    return {
        "iota_bc": ((128, 128), BF16, "ExternalInput"),
        "b1_bc": ((128, 128), F32, "ExternalInput"),
        "ri_pc": ((128, n_chunks), F32, "ExternalInput"),
        "dv1": ((128, p["tot_tiles"]), BF16, "ExternalInput"),
        "idxh": ((128, 2 * p["n_groups"] * p["Tmax"] * 8), I16, "ExternalInput"),
        "hq": ((n, D), BF16, "ExternalInput"),
        "out": ((n_chunks * 128, D), F32, "ExternalOutput"),
    }


def in_maps0(p):
    c = p["consts"]
    return [
        {"iota_bc": c["iota_bc"], "W0p4": c["W0p4"], "W1": c["W1"],
         "dv0": p["dv0"][i], "rodT": p["rodT"][i], "x0h": p["x0h"][i]}
        for i in range(NC_CORES)
    ]


def in_maps1(p, hq_full):
    c = p["consts"]
    return [
        {"iota_bc": c["iota_bc"], "b1_bc": c["b1_bc"], "ri_pc": p["ri_pc"][i],
         "dv1": p["dv1"][i],
         "idxh": np.ascontiguousarray(
             p["idxh"][i].reshape(2 * p["n_groups"], 128, p["Tmax"] * 8)
             .transpose(1, 0, 2).reshape(128, -1)),
         "hq": hq_full}
        for i in range(NC_CORES)
    ]


def _build(body, tensors, nq=1, **kw):
    nc = _new_nc(nq)
    aps = {
        name: nc.dram_tensor(name, list(shape), dtype, kind=kind).ap()
        for name, (shape, dtype, kind) in tensors.items()
    }
    with tile.TileContext(nc) as tc:
        body(tc, aps, **kw)
    nc.compile()
    return nc


class _SimResults:
    def __init__(self, results):
        self.results = results
        self.exec_time_ns = None


def _run_sim(nc, in_maps, out_names):
    from concourse.bass_interp import CoreSim
    results = []
    for im in in_maps:
        sim = CoreSim(nc)
        for k, v in im.items():
            sim.tensor(k)[:] = v
        sim.simulate(check_with_hw=False)
        results.append({k: np.array(sim.tensor(k)) for k in out_names})
    return _SimResults(results)


# --------------------------------------------------------------------------
# entry point
# --------------------------------------------------------------------------

def _to_bf16(a):
    import ml_dtypes
    return np.asarray(a).astype(ml_dtypes.bfloat16)


def kernel(src, dst, weight, significance, emb, W0, b0, W1, b1):
    global LAST_EXEC_TIMES_NS, LAST_RESULTS
    LAST_EXEC_TIMES_NS = []
    LAST_RESULTS = []
    trace = bool(os.environ.get("BASS_TRACE"))

    p = _prep(src, dst, weight, significance, emb, W0, b0, W1, b1)
    n, npc, n_chunks = p["n"], p["npc"], p["n_chunks"]
    c = p["consts"]

    # bf16-cast inputs where specs say BF16
    c["iota_bc"] = _to_bf16(c["iota_bc"])
    c["W0p4"] = _to_bf16(c["W0p4"])
    c["W1"] = _to_bf16(c["W1"])
    p["dv0"] = _to_bf16(p["dv0"])
    p["x0h"] = _to_bf16(p["x0h"])
    p["dv1"] = _to_bf16(p["dv1"])

    use_sim = bool(os.environ.get("BASS_SIM"))

    nc0 = _build(_conv0_body, tensor_specs0(p), n_chunks=n_chunks,
                 t0k=p["t0k"], x0off=p["x0off"], tot0=p["tot0"])
    if use_sim:
        res0 = _run_sim(nc0, in_maps0(p), ["hq"])
    else:
        res0 = run_bass_kernel_spmd(nc0, in_maps0(p),
                                    core_ids=list(range(NC_CORES)), trace=trace)
    LAST_RESULTS.append(res0)
    LAST_EXEC_TIMES_NS.append(res0.exec_time_ns)
    hq_full = np.concatenate(
        [np.asarray(res0.results[i]["hq"][:npc]) for i in range(NC_CORES)], axis=0
    )
    assert hq_full.shape == (n, D)

    # patch zero-in-degree nodes (b0 fold drops them): hq = leaky(b0)*od @ W1
    zid = np.nonzero(p["in_deg"] == 0)[0]
    if zid.size:
        b0f = p["b0"]
        v = np.where(b0f > 0, b0f, NEG_SLOPE * b0f)[None, :] * p["od"][zid][:, None]
        hq_full[zid] = _to_bf16(v @ p["W1f"])

    nc1 = _build(_conv1_body, tensor_specs1(p), nq=NQUEUES, p=p)
    if use_sim:
        res1 = _run_sim(nc1, in_maps1(p, hq_full), ["out"])
    else:
        res1 = run_bass_kernel_spmd(nc1, in_maps1(p, hq_full),
                                    core_ids=list(range(NC_CORES)), trace=trace)
    LAST_RESULTS.append(res1)
    LAST_EXEC_TIMES_NS.append(res1.exec_time_ns)

    out = np.concatenate(
        [res1.results[i]["out"][:npc] for i in range(NC_CORES)], axis=0
    )
    assert out.shape == (n, D)
    return out.astype(np.float32)
